# revision 11
# baseline (speedup 1.0000x reference)
"""KL-divergence loss kernel (C51 categorical projection + batchmean KL) for TRN2.

Math: the reference projects `anchor` through a C51 projection whose skew is a
compile-time scalar, so the projection collapses to a constant linear map:

    t[:, 0]  = 0
    t[:, 1]  = 0.75*a[:, 0]
    t[:, j]  = 0.75*a[:, j-1] + 0.25*a[:, j-2]          (2 <= j <= 49)
    t[:, 50] = 0.25*a[:, 48] + a[:, 49] + a[:, 50]

and the loss is sum(t * (log t - log(f + 1e-16))) / B  (terms with t==0 are 0).

Kernel strategy (pure data parallel over 8 cores, batch-sharded):
  s = 4t built with one wide fused scalar_tensor_tensor (s_j = 3*a_{j-1} + a_{j-2})
  lt = Ln(0.25*s + 1e-35)      [ScalarE, fused affine, bf16 out]
  lf = Ln(f + 1e-16)           [ScalarE, fused affine, bf16 out]
  lt and lf land in two halves of one fused SBUF tile; TensorE matmuls
  lhsT=s_blk against rhs spanning BOTH halves (free dim 2*w) so one PSUM
  accumulator [128, 256] collects sum(s*lt) on diag cells [j, j] and
  sum(s*lf) on [j, 128+j].  Host sums diag(lt-half) - diag(lf-half) over
  the 8 per-core results and scales by 0.25/B.

  The ScalarE tail (Ln work that can only run after the final bytes land)
  is bounded by the "phase lag" between a tile's bytes landing and its Ln
  inputs becoming ready, which scales with the final transfer sizes and
  the DVE s-build latency.  So the schedule tapers to tiny tiles at the
  end ([...,40,16,8]), the tail tiles' stt/Ln(s)/Ln(f) are half-split, and
  the last two feature transfers are issued after the last two anchor
  transfers so the stream ends on the bytes that unlock the least work.
"""

import os
import numpy as np

B_TOTAL = 524288
ATOMS = 51
N_CORES = 8
ROWS_PER_CORE = B_TOTAL // N_CORES  # 65536
P = 128
R_SCHED = [64, 64, 64, 64, 64, 64, 64, 40, 16, 8]
assert sum(R_SCHED) * P == ROWS_PER_CORE
N_TAIL = 3  # last N_TAIL tiles get half-split stt/Ln and the a/f reorder
MM_BLOCK = 128

_BUILT = None
_LAST_RESULTS = None


def _blocks(cols, edges):
    """128-wide matmul blocks, additionally cut at the given column edges."""
    cuts = sorted(set([0, cols] + [e for e in edges if 0 < e < cols]))
    out = []
    for lo, hi in zip(cuts[:-1], cuts[1:]):
        c = lo
        while c < hi:
            w = min(MM_BLOCK, hi - c)
            out.append((c, w))
            c += w
    return out


def _build():
    from contextlib import ExitStack

    import concourse.bacc as bacc
    import concourse.tile as tile
    from concourse import mybir

    nc = bacc.Bacc("TRN2", num_devices=N_CORES)

    a_dram = nc.dram_tensor(
        "anchor", [ROWS_PER_CORE, ATOMS], mybir.dt.float32, kind="ExternalInput"
    )
    f_dram = nc.dram_tensor(
        "feature", [ROWS_PER_CORE, ATOMS], mybir.dt.float32, kind="ExternalInput"
    )
    out_dram = nc.dram_tensor(
        "out", [P, 2 * MM_BLOCK], mybir.dt.float32, kind="ExternalOutput"
    )

    mult = mybir.AluOpType.mult
    add = mybir.AluOpType.add
    Ln = mybir.ActivationFunctionType.Ln

    n_tiles = len(R_SCHED)
    tail0 = n_tiles - N_TAIL

    total_mms = 0
    for i, R in enumerate(R_SCHED):
        cols = R * ATOMS
        edges = [cols // 2] if i == tail0 else []
        total_mms += len(_blocks(cols, edges))

    with tile.TileContext(nc) as tc:
        with ExitStack() as ctx:
            a_pool = ctx.enter_context(tc.tile_pool(name="a", bufs=3))
            f_pool = ctx.enter_context(tc.tile_pool(name="f", bufs=3))
            s_pool = ctx.enter_context(tc.tile_pool(name="s", bufs=3))
            ll_pool = ctx.enter_context(tc.tile_pool(name="ll", bufs=3))
            tmp_pool = ctx.enter_context(tc.tile_pool(name="tmp", bufs=4))
            out_pool = ctx.enter_context(tc.tile_pool(name="outp", bufs=1))
            psum_pool = ctx.enter_context(
                tc.tile_pool(name="acc", bufs=1, space="PSUM")
            )

            acc = psum_pool.tile([P, 2 * MM_BLOCK], mybir.dt.float32)
            acc2 = acc[:].rearrange("m (two c) -> m two c", two=2)

            eps_t = out_pool.tile([P, 1], mybir.dt.float32, tag="eps_t")
            eps_f = out_pool.tile([P, 1], mybir.dt.float32, tag="eps_f")
            warm = out_pool.tile([P, 1], mybir.dt.float32, tag="warm")
            nc.gpsimd.memset(eps_t[:], 1e-35)
            nc.gpsimd.memset(eps_f[:], 1e-16)
            # dummy activation: hoists the ~1.3us ACT_TABLE_LOAD off the
            # critical path (otherwise it lands right before the first real Ln)
            nc.scalar.activation(
                out=warm[:], in_=eps_f[:], func=Ln, bias=eps_f[:], scale=1.0
            )

            def build_s(s_sb, a_sb, R, qlo, qhi):
                """s rows [qlo, qhi) from a (strided per-atom form)."""
                a3 = a_sb[:].rearrange("p (q m) -> p q m", m=ATOMS)
                s3 = s_sb[:].rearrange("p (q m) -> p q m", m=ATOMS)
                nc.gpsimd.memset(s3[:, qlo:qhi, 0], 0.0)
                nc.vector.scalar_tensor_tensor(
                    out=s3[:, qlo:qhi, 2:50],
                    in0=a3[:, qlo:qhi, 1:49],
                    scalar=3.0,
                    in1=a3[:, qlo:qhi, 0:48],
                    op0=mult,
                    op1=add,
                )
                nc.vector.tensor_scalar_mul(
                    s3[:, qlo:qhi, 1], a3[:, qlo:qhi, 0], 3.0
                )
                tmp = tmp_pool.tile([P, 64], mybir.dt.float32)
                nc.vector.scalar_tensor_tensor(
                    out=tmp[:, 0 : qhi - qlo],
                    in0=a3[:, qlo:qhi, 49],
                    scalar=4.0,
                    in1=a3[:, qlo:qhi, 48],
                    op0=mult,
                    op1=add,
                )
                nc.vector.scalar_tensor_tensor(
                    out=s3[:, qlo:qhi, 50],
                    in0=a3[:, qlo:qhi, 50],
                    scalar=4.0,
                    in1=tmp[:, 0 : qhi - qlo],
                    op0=mult,
                    op1=add,
                )

            # allocate tail tiles' buffers up front so the final two feature
            # DMAs can be issued after the final anchor DMAs
            tiles = {}
            for i, R in enumerate(R_SCHED):
                cols = R * ATOMS
                tiles[i] = dict(
                    R=R,
                    cols=cols,
                    a=None,
                    f=None,
                    s=None,
                    ll=None,
                )

            mm = 0
            r0_rows = [0]
            for R in R_SCHED:
                r0_rows.append(r0_rows[-1] + P * R)

            def a_src(i):
                r0 = r0_rows[i]
                return (
                    a_dram.ap()[r0 : r0 + P * R_SCHED[i], :]
                    .rearrange("(p q) m -> p (q m)", p=P)
                )

            def f_src3(i):
                r0 = r0_rows[i]
                return (
                    f_dram.ap()[r0 : r0 + P * R_SCHED[i], :]
                    .rearrange("(p q) m -> p q m", p=P)
                )

            def emit_a(i):
                t = tiles[i]
                t["a"] = a_pool.tile([P, t["cols"]], mybir.dt.float32, name="a_sb", tag="a_sb")
                nc.sync.dma_start(out=t["a"][:], in_=a_src(i))

            def emit_f(i, halves):
                t = tiles[i]
                t["f"] = f_pool.tile([P, t["cols"]], mybir.dt.float32, name="f_sb", tag="f_sb")
                if halves:
                    h = t["R"] // 2
                    hc = h * ATOMS
                    nc.sync.dma_start(
                        out=t["f"][:, 0:hc], in_=f_src3(i)[:, 0:h, :]
                    )
                    nc.sync.dma_start(
                        out=t["f"][:, hc : t["cols"]],
                        in_=f_src3(i)[:, h : t["R"], :],
                    )
                else:
                    nc.sync.dma_start(
                        out=t["f"][:], in_=f_src3(i).rearrange("p q m -> p (q m)")
                    )

            def emit_compute(i, split):
                nonlocal mm
                t = tiles[i]
                R, cols = t["R"], t["cols"]
                t["s"] = s_pool.tile([P, cols], mybir.dt.bfloat16, name="s_sb", tag="s_sb")
                t["ll"] = ll_pool.tile([P, 2 * cols], mybir.dt.bfloat16, name="ll_sb", tag="ll_sb")
                s_sb, ll_sb, a_sb, f_sb = t["s"], t["ll"], t["a"], t["f"]
                if split:
                    h = R // 2
                    hc = h * ATOMS
                    build_s(s_sb, a_sb, R, 0, h)
                    nc.scalar.activation(
                        out=ll_sb[:, 0:hc], in_=s_sb[:, 0:hc],
                        func=Ln, bias=eps_t[:], scale=0.25,
                    )
                    build_s(s_sb, a_sb, R, h, R)
                    nc.scalar.activation(
                        out=ll_sb[:, hc:cols], in_=s_sb[:, hc:cols],
                        func=Ln, bias=eps_t[:], scale=0.25,
                    )
                    nc.scalar.activation(
                        out=ll_sb[:, cols : cols + hc], in_=f_sb[:, 0:hc],
                        func=Ln, bias=eps_f[:], scale=1.0,
                    )
                    nc.scalar.activation(
                        out=ll_sb[:, cols + hc : 2 * cols], in_=f_sb[:, hc:cols],
                        func=Ln, bias=eps_f[:], scale=1.0,
                    )
                else:
                    build_s(s_sb, a_sb, R, 0, R)
                    nc.scalar.activation(
                        out=ll_sb[:, 0:cols], in_=s_sb[:],
                        func=Ln, bias=eps_t[:], scale=0.25,
                    )
                    nc.scalar.activation(
                        out=ll_sb[:, cols : 2 * cols], in_=f_sb[:],
                        func=Ln, bias=eps_f[:], scale=1.0,
                    )
                ll2 = ll_sb[:].rearrange("p (two c) -> p two c", two=2)
                edges = [cols // 2] if split else []
                for c0, w in _blocks(cols, edges):
                    nc.tensor.matmul(
                        acc2[0:w, :, 0:w],
                        s_sb[:, c0 : c0 + w],
                        ll2[:, :, c0 : c0 + w],
                        start=(mm == 0),
                        stop=(mm == total_mms - 1),
                    )
                    mm += 1

            # bulk tiles: a_i, f_i interleaved (the proven stream pattern)
            for i in range(tail0):
                emit_a(i)
                emit_f(i, halves=False)
                emit_compute(i, split=False)
            # tail tiles: issue a-transfers first, then f (smallest-work
            # bytes last); tile tail0 keeps f-halves, the tiny ones don't
            for i in range(tail0, n_tiles):
                emit_a(i)
            emit_f(tail0, halves=True)
            for i in range(tail0 + 1, n_tiles):
                emit_f(i, halves=False)
            for i in range(tail0, n_tiles):
                emit_compute(i, split=(i == tail0))

            out_sb = out_pool.tile([P, 2 * MM_BLOCK], mybir.dt.float32)
            nc.vector.tensor_copy(out_sb[:], acc[:])
            nc.sync.dma_start(out=out_dram.ap(), in_=out_sb[:])

    nc.compile()
    return nc


def kernel(anchor: np.ndarray, feature: np.ndarray) -> np.ndarray:
    global _BUILT, _LAST_RESULTS
    from concourse import bass_utils

    if _BUILT is None:
        _BUILT = _build()
    nc = _BUILT

    anchor = np.ascontiguousarray(anchor, dtype=np.float32)
    feature = np.ascontiguousarray(feature, dtype=np.float32)

    in_maps = []
    for c in range(N_CORES):
        lo, hi = c * ROWS_PER_CORE, (c + 1) * ROWS_PER_CORE
        in_maps.append({"anchor": anchor[lo:hi], "feature": feature[lo:hi]})

    res = bass_utils.run_bass_kernel_spmd(
        nc,
        in_maps,
        core_ids=list(range(N_CORES)),
        trace=bool(os.environ.get("BASS_TRACE")),
    )
    _LAST_RESULTS = res

    total = 0.0
    for c in range(N_CORES):
        out = res.results[c]["out"].astype(np.float64)
        total += np.trace(out[:, :MM_BLOCK]) - np.trace(out[:, MM_BLOCK:])
    val = 0.25 * total / B_TOTAL
    return np.array(val, dtype=np.float32)
